# revision 14
# baseline (speedup 1.0000x reference)
"""Trainium2 Bass kernel for nn_DecoderRNN (GRU decoder, 140 sequential steps).

Strategy (data-parallel, per sharding hint): B=512 sharded 8 ways -> 64
batch rows/core; weights replicated. Feature-major on-chip layout
([128 part, chunk*64+batch] free).

v2 redesign vs baseline (1.33 ms):
  - Weo fusion: e_t = relu(Weo h_t + b_eo) with Weo = emb_W @ out_W
    precomputed on host; the out-projection/x intermediate disappears from
    the recurrence. Decoder outputs come from regWo = reg_W @ out_W.
  - Gate-major PSUM banks: E|R in bank A, Z|HN in bank B, IN|Y in bank C
    (each gate region [128, 4*64]). One wide ACT/DVE op per gate instead
    of 4 narrow ones (per-op overhead dominated at [128,64]).
  - Biases pre-added into PSUM with one K=4 "indicator" matmul per gate
    region (stationary = 4 bias chunks as rows, moving = 0/1 selector).
    Frees ACT ops of per-chunk bias and doubles as PE filler.
  - h' = (1-z)*n + z*h with omz/zh computed on DVE during the tanh window
    -> only 2 DVE ops after tanh on the critical path.
  - PE stream ordered: chain matmuls first, then y-projections / next
    step's bias MMs / dummy warmers as filler so the PE never idles long
    enough for HAM to re-throttle the clock to 1.2 GHz (baseline spent
    73% of the kernel at half clock).
"""

import numpy as np
import ml_dtypes

B, T_ENC, E, H, O, PRED_LEN = 512, 140, 256, 512, 64, 140
NCORES = 8
BC = B // NCORES           # 64 batch rows per core
T_ALL = T_ENC + PRED_LEN   # 280

bf16 = ml_dtypes.bfloat16


def _pack_tiles(wT, n_k, n_m):
    """Pack a [K, M] (pre-transposed) weight into [128, n_m*n_k*128] bf16:
    tile (mi, k) at cols (mi*n_k + k)*128."""
    K, M = wT.shape
    assert K == n_k * 128 and M == n_m * 128
    t = wT.reshape(n_k, 128, n_m, 128).transpose(2, 0, 1, 3)  # [mc, kc, 128, 128]
    t = t.transpose(2, 0, 1, 3).reshape(128, -1)
    return np.ascontiguousarray(t.astype(bf16))


def _feat_major(x, n_chunks):
    """[B, F] -> [128, n_chunks*B] feature-major chunk layout."""
    b, f = x.shape
    assert f == n_chunks * 128
    t = x.reshape(b, n_chunks, 128).transpose(2, 1, 0).reshape(128, n_chunks * b)
    return np.ascontiguousarray(t)


def build_program(nsteps=PRED_LEN, t_enc=T_ENC, lowering=True):
    """Build the Bass program (per-core SPMD). Returns nc."""
    import concourse.bass as bass
    import concourse.tile as tile
    from concourse import bacc, mybir

    AF = mybir.ActivationFunctionType
    OP = mybir.AluOpType
    f32 = mybir.dt.float32
    bf = mybir.dt.bfloat16

    t_all = t_enc + nsteps

    if lowering:
        nc = bacc.Bacc("TRN2", target_bir_lowering=True, debug=False)
    else:
        nc = bass.Bass("TRN2", target_bir_lowering=False, debug=False)

    # ---- DRAM I/O ----
    encT_d = nc.dram_tensor("encT", [128, t_enc * 128], bf, kind="ExternalInput").ap()
    h0_d = nc.dram_tensor("h0T", [128, 4 * BC], bf, kind="ExternalInput").ap()
    wih_d = nc.dram_tensor("wihT", [128, 48 * 128], bf, kind="ExternalInput").ap()
    whh_d = nc.dram_tensor("whhT", [128, 48 * 128], bf, kind="ExternalInput").ap()
    weo_d = nc.dram_tensor("weoT", [128, 16 * 128], bf, kind="ExternalInput").ap()
    emb_d = nc.dram_tensor("embT", [128, 8 * 128], bf, kind="ExternalInput").ap()
    regw_d = nc.dram_tensor("regwT", [128, 2 * O], bf, kind="ExternalInput").ap()
    regwo_d = nc.dram_tensor("regwoT", [128, 4 * O], bf, kind="ExternalInput").ap()
    biasst_d = nc.dram_tensor("biasst", [4, 6 * 128], bf, kind="ExternalInput").ap()
    ind_d = nc.dram_tensor("ind", [4, 256], bf, kind="ExternalInput").ap()
    breg_d = nc.dram_tensor("b_reg", [BC, O], f32, kind="ExternalInput").ap()
    bregd_d = nc.dram_tensor("b_regd", [BC, O], f32, kind="ExternalInput").ap()
    y_d = nc.dram_tensor("y", [BC, t_all, O], f32, kind="ExternalOutput").ap()
    dbg_d = nc.dram_tensor("dbg", [1, 1], f32, kind="ExternalOutput").ap()

    # bias bank order in biasst: E0(b_e), E1(b_eo), R, Z, HN, IN
    BE0, BE1, BR, BZ, BHN, BIN = range(6)

    with tile.TileContext(nc) as tc:
        import contextlib
        with contextlib.ExitStack() as ctx:
            consts = ctx.enter_context(tc.tile_pool(name="consts", bufs=1))
            temps = ctx.enter_context(tc.tile_pool(name="temps", bufs=3))
            ytmp = ctx.enter_context(tc.tile_pool(name="ytmp", bufs=4))
            psum = ctx.enter_context(tc.tile_pool(name="psum", bufs=2, space="PSUM"))

            # ---- ACT table warmup (walrus inserts the table load before the
            # first ACTIVATE; pin it to dependency-light dummy ops) ----
            wt = consts.tile([128, 8], f32, tag="wtbl", name="wtbl")
            nc.vector.memset(wt[:, 0:4], 0.0)
            nc.scalar.activation(wt[:, 4:5], wt[:, 0:1], AF.Relu)
            nc.scalar.activation(wt[:, 5:6], wt[:, 1:2], AF.Sigmoid)
            nc.scalar.activation(wt[:, 6:7], wt[:, 2:3], AF.Tanh)

            # ---- load constants into SBUF ----
            wih_sb = consts.tile([128, 48 * 128], bf, tag="wih")
            whh_sb = consts.tile([128, 48 * 128], bf, tag="whh")
            weo_sb = consts.tile([128, 16 * 128], bf, tag="weo")
            emb_sb = consts.tile([128, 8 * 128], bf, tag="emb")
            regw_sb = consts.tile([128, 2 * O], bf, tag="regw")
            regwo_sb = consts.tile([128, 4 * O], bf, tag="regwo")
            biasst_sb = consts.tile([4, 6 * 128], bf, tag="biasst")
            ind_sb = consts.tile([4, 256], bf, tag="ind")
            breg_sb = consts.tile([BC, O], f32, tag="breg")
            bregd_sb = consts.tile([BC, O], f32, tag="bregd")
            encT_sb = consts.tile([128, t_enc * 128], bf, tag="encT")

            nc.sync.dma_start(out=biasst_sb, in_=biasst_d)
            nc.sync.dma_start(out=ind_sb, in_=ind_d)
            nc.sync.dma_start(out=emb_sb, in_=emb_d)
            # x0 block (last encoder token) first so step 0 can start early
            lastblk = slice((t_enc - 1) * 128, t_enc * 128)
            nc.sync.dma_start(out=encT_sb[:, lastblk], in_=encT_d[:, lastblk])
            nc.sync.dma_start(out=whh_sb, in_=whh_d)
            nc.sync.dma_start(out=wih_sb, in_=wih_d)
            nc.sync.dma_start(out=weo_sb, in_=weo_d)
            nc.sync.dma_start(out=regw_sb, in_=regw_d)
            nc.sync.dma_start(out=regwo_sb, in_=regwo_d)
            nc.sync.dma_start(out=breg_sb, in_=breg_d)
            nc.sync.dma_start(out=bregd_sb, in_=bregd_d)
            # rest of encT in 4 chunks
            nsplit = 4
            per = (t_enc - 1) // nsplit + 1
            for i in range(nsplit):
                lo, hi = i * per, min((i + 1) * per, t_enc - 1)
                if lo >= hi:
                    continue
                nc.sync.dma_start(out=encT_sb[:, lo * 128:hi * 128],
                                  in_=encT_d[:, lo * 128:hi * 128])

            # ---- persistent state: h ping-pong [128, 4*BC] ----
            h_pp = [consts.tile([128, 4 * BC], bf, tag=f"h{s}", name=f"h{s}")
                    for s in range(2)]
            nc.sync.dma_start(out=h_pp[0], in_=h0_d)

            def wtile(sb, mi, k, n_k):
                j = (mi * n_k + k) * 128
                return sb[:, j:j + 128]

            def bias_mm(region, bank):
                st = biasst_sb[:, bank * 128:(bank + 1) * 128]
                nc.tensor.matmul(region, st, ind_sb, start=True, stop=False)

            def alloc_banks(t, n_dummy=0):
                """Allocate step-t PSUM banks; issue HAM-warmer dummies (into
                the E corner, pre-bias so they get overwritten) + bias MMs.

                Separate tiles per gate region so no gate read shares a tile
                with unrelated later writers (keeps the auto-semaphores
                tight). Y projections ride in the Z tile's tail columns.
                """
                psA = psum.tile([128, 512], f32, tag="A", name=f"psA{t % 2}")
                psB = psum.tile([128, 512], f32, tag="B", name=f"psB{t % 2}")
                psHN = psB[:, 0:256]
                psIN = psB[:, 256:512]
                psZY = psum.tile([128, 384], f32, tag="ZY", name=f"psZY{t % 2}")
                for _ in range(n_dummy):
                    # keeps the PE's activity monitor fed during the gate
                    # window (else it re-throttles the clock to 1.2 GHz)
                    nc.tensor.matmul(psA[0:1, 0:64], whh_sb[:, 0:1],
                                     encT_sb[:, 0:64], start=True, stop=True)
                bias_mm(psA[:, 0:256], BE0 if t == 0 else BE1)   # E
                bias_mm(psA[:, 256:512], BR)                     # R
                bias_mm(psZY[:, 0:256], BZ)                      # Z
                bias_mm(psHN, BHN)                               # HN
                bias_mm(psIN, BIN)                               # IN
                return psA, psHN, psIN, psZY

            banks = alloc_banks(0)

            for t in range(nsteps):
                p, pn = t % 2, (t + 1) % 2
                h_cur = h_pp[p]
                psA, psHN, psIN, psZY = banks

                # ---- dense PE block (chain order) ----
                # 1) E: e pre-act = Weo h + b_eo  (t=0: emb x0 + b_e)
                if t == 0:
                    for m in range(4):
                        for k in range(2):
                            nc.tensor.matmul(
                                psA[:, m * BC:(m + 1) * BC],
                                wtile(emb_sb, m, k, 2),
                                encT_sb[:, (t_enc - 1) * 128 + k * BC:
                                        (t_enc - 1) * 128 + (k + 1) * BC],
                                start=False, stop=(k == 1))
                else:
                    for m in range(4):
                        for k in range(4):
                            nc.tensor.matmul(
                                psA[:, m * BC:(m + 1) * BC],
                                wtile(weo_sb, m, k, 4),
                                h_cur[:, k * BC:(k + 1) * BC],
                                start=False, stop=(k == 3))
                # 2) e: one wide relu (bias already in PSUM via bias MM)
                e = temps.tile([128, 256], bf, tag="e")
                nc.scalar.activation(e, psA[:, 0:256], AF.Relu)
                # 3) gate regions ordered by remaining-chain length:
                #    R first (r -> t3 -> t4 -> tanh -> q -> h'), then HN (t3),
                #    IN (t4), Z last (z -> omz/zh -> q/h').
                def whh_mms(base, ps, col0, stop):
                    for m in range(4):
                        for k in range(4):
                            nc.tensor.matmul(
                                ps[:, col0 + m * BC:col0 + (m + 1) * BC],
                                wtile(whh_sb, base + m, k, 4),
                                h_cur[:, k * BC:(k + 1) * BC],
                                start=False, stop=(stop and k == 3))

                def wih_mms(base, ps, col0):
                    # k-outer: the last e chunk only gates 4 trailing MMs
                    for k in range(4):
                        for m in range(4):
                            nc.tensor.matmul(
                                ps[:, col0 + m * BC:col0 + (m + 1) * BC],
                                wtile(wih_sb, base + m, k, 4),
                                e[:, k * BC:(k + 1) * BC],
                                start=False, stop=(k == 3))

                whh_mms(0, psA, 256, False)    # R (whh part)
                wih_mms(0, psA, 256)           # R (wih part) - waits on e
                whh_mms(8, psHN, 0, True)      # HN (complete)
                wih_mms(8, psIN, 0)            # IN
                whh_mms(4, psZY, 0, False)     # Z (whh part)
                wih_mms(4, psZY, 0)            # Z (wih part)

                # ---- PE filler MMs: y projections, next banks' bias MMs,
                # warmers. No pending deps -> they execute while ACT/DVE run
                # the gate chain. Their DVE/DMA tails are emitted after the
                # gate ops so they can't head-of-line-block the DVE queue.
                if t > 0:
                    for k in range(4):
                        nc.tensor.matmul(psZY[:BC, 320:384],
                                         h_cur[:, k * BC:(k + 1) * BC],
                                         regwo_sb[:, k * O:(k + 1) * O],
                                         start=(k == 0), stop=(k == 3))
                if t < t_enc:
                    for k in range(2):
                        nc.tensor.matmul(psZY[:BC, 256:320],
                                         encT_sb[:, t * 128 + k * BC:
                                                 t * 128 + (k + 1) * BC],
                                         regw_sb[:, k * O:(k + 1) * O],
                                         start=(k == 0), stop=(k == 1))
                if t + 1 < nsteps:
                    banks = alloc_banks(t + 1, n_dummy=2)

                # ---- gate chain (ACT / DVE / GpSimd) ----
                r = temps.tile([128, 256], bf, tag="r")
                z = temps.tile([128, 256], bf, tag="z")
                t3 = temps.tile([128, 256], bf, tag="t3")
                t4 = temps.tile([128, 256], bf, tag="t4")
                n_t = temps.tile([128, 256], bf, tag="n")
                omz = temps.tile([128, 256], bf, tag="omz")
                zh = temps.tile([128, 256], bf, tag="zh")
                q = temps.tile([128, 256], bf, tag="q")
                h_nxt = h_pp[pn]

                # ACT queue: relu(above), r, tanh, z
                nc.scalar.activation(r, psA[:, 256:512], AF.Sigmoid)
                nc.vector.tensor_tensor(t3, psHN, r, OP.mult)
                nc.vector.tensor_tensor(t4, t3, psIN, OP.add)
                nc.scalar.activation(n_t, t4, AF.Tanh)
                nc.scalar.activation(z, psZY[:, 0:256], AF.Sigmoid)
                nc.vector.tensor_scalar(omz, z, -1.0, 1.0, OP.mult, OP.add)
                nc.gpsimd.tensor_tensor(zh, z, h_cur, OP.mult)
                nc.vector.tensor_tensor(q, omz, n_t, OP.mult)
                nc.vector.tensor_tensor(h_nxt, q, zh, OP.add)

                # chain-gated HAM warmers: each becomes ready only as the
                # gate chain progresses, spreading PE activity through the
                # tail so the activity monitor never sees an idle window.
                # They write row 0 of the E region (already consumed by
                # relu; next step's bias MM start=True overwrites).
                for src_t in (r, t3, t4, z, n_t, q):
                    nc.tensor.matmul(psA[0:1, 0:256], src_t[:, 0:1],
                                     encT_sb[:, 0:256], start=True, stop=True)

                # ---- y staging (DVE) + DMA, after the gate chain on DVE ----
                if t > 0:
                    y_dec = ytmp.tile([BC, O], f32, tag="ydec")
                    nc.vector.tensor_tensor(y_dec, psZY[:BC, 320:384],
                                            bregd_sb, OP.add)
                    nc.sync.dma_start(out=y_d[:, t_enc + t - 1, :], in_=y_dec)
                if t < t_enc:
                    y_enc = ytmp.tile([BC, O], f32, tag="yenc")
                    nc.vector.tensor_tensor(y_enc, psZY[:BC, 256:320],
                                            breg_sb, OP.add)
                    nc.sync.dma_start(out=y_d[:, t, :], in_=y_enc)

            # final decoder token: dec-y for h_{nsteps} -> y[t_enc+nsteps-1]
            h_last = h_pp[nsteps % 2]
            ps_fin = psum.tile([128, 512], f32, tag="A", name="psfin")
            for k in range(4):
                nc.tensor.matmul(ps_fin[:BC, 320:384],
                                 h_last[:, k * BC:(k + 1) * BC],
                                 regwo_sb[:, k * O:(k + 1) * O],
                                 start=(k == 0), stop=(k == 3))
            y_fin = ytmp.tile([BC, O], f32, tag="ydec")
            nc.vector.tensor_tensor(y_fin, ps_fin[:BC, 320:384], bregd_sb, OP.add)
            nc.sync.dma_start(out=y_d[:, t_enc + nsteps - 1, :], in_=y_fin)

            # leftover encoder tokens if nsteps < t_enc (smoke tests only)
            for t in range(nsteps, t_enc):
                ps_y2 = psum.tile([128, 512], f32, tag="A", name=f"psy{t}")
                for k in range(2):
                    nc.tensor.matmul(ps_y2[:BC, 256:320],
                                     encT_sb[:, t * 128 + k * BC:
                                             t * 128 + (k + 1) * BC],
                                     regw_sb[:, k * O:(k + 1) * O],
                                     start=(k == 0), stop=(k == 1))
                y_enc = ytmp.tile([BC, O], f32, tag="yenc")
                nc.vector.tensor_tensor(y_enc, ps_y2[:BC, 256:320], breg_sb, OP.add)
                nc.sync.dma_start(out=y_d[:, t, :], in_=y_enc)

            # anti-DCE sink for the warmer scratch
            ps_wm = psum.tile([128, 256], f32, tag="wm", bufs=1, name="wmfin")
            nc.tensor.matmul(ps_wm, whh_sb[:, 0:128], encT_sb[:, 0:256],
                             start=True, stop=True)
            wm_sb = ytmp.tile([1, 1], f32, tag="wmsb")
            nc.vector.tensor_copy(wm_sb, ps_wm[0:1, 0:1])
            nc.sync.dma_start(out=dbg_d, in_=wm_sb)

    if lowering:
        nc.finalize()
    return nc


def prep_inputs(encoder_outputs, encoder_hidden, emb_W, emb_b, w_ih, w_hh,
                b_ih, b_hh, out_W, out_b, reg_W, reg_b, nsteps=PRED_LEN,
                t_enc=T_ENC):
    """Host-side packing. Returns per-core input dicts."""
    f32 = np.float32
    emb_W, emb_b, w_ih, w_hh, b_ih, b_hh, out_W, out_b, reg_W, reg_b = (
        np.asarray(a, f32) for a in
        (emb_W, emb_b, w_ih, w_hh, b_ih, b_hh, out_W, out_b, reg_W, reg_b))

    weo = emb_W @ out_W            # [H, H]
    b_eo = emb_W @ out_b + emb_b   # [H]
    regwo = reg_W @ out_W          # [O, H]
    b_regd = reg_W @ out_b + reg_b # [O]

    # bias stationary [4, 6*128]: rows = feature chunks; bank order
    # E0(b_e), E1(b_eo), R, Z, HN, IN
    b_r = b_ih[0:H] + b_hh[0:H]
    b_z = b_ih[H:2 * H] + b_hh[H:2 * H]
    b_hn = b_hh[2 * H:]
    b_in = b_ih[2 * H:]
    biasst = np.stack([emb_b, b_eo, b_r, b_z, b_hn, b_in], 0)  # [6, 512]
    biasst = (biasst.reshape(6, 4, 128).transpose(1, 0, 2)
              .reshape(4, 6 * 128))
    ind = np.zeros((4, 256), f32)
    for c in range(4):
        ind[c, c * BC:(c + 1) * BC] = 1.0

    shared = {
        "wihT": _pack_tiles(w_ih.T, 4, 12),
        "whhT": _pack_tiles(w_hh.T, 4, 12),
        "weoT": _pack_tiles(weo.T, 4, 4),
        "embT": _pack_tiles(emb_W.T, 2, 4),
        "regwT": np.ascontiguousarray(
            reg_W.T.reshape(2, 128, O).transpose(1, 0, 2).reshape(128, 2 * O)
            .astype(bf16)),
        "regwoT": np.ascontiguousarray(
            regwo.T.reshape(4, 128, O).transpose(1, 0, 2).reshape(128, 4 * O)
            .astype(bf16)),
        "biasst": np.ascontiguousarray(biasst.astype(bf16)),
        "ind": np.ascontiguousarray(ind.astype(bf16)),
        "b_reg": np.ascontiguousarray(np.tile(reg_b[None, :], (BC, 1)).astype(f32)),
        "b_regd": np.ascontiguousarray(np.tile(b_regd[None, :], (BC, 1)).astype(f32)),
    }

    enc = np.asarray(encoder_outputs, f32)[:, :t_enc, :]
    h0 = np.asarray(encoder_hidden, f32)[0]
    in_maps = []
    for i in range(NCORES):
        sl = slice(i * BC, (i + 1) * BC)
        enc_i = enc[sl].astype(bf16)              # [BC, t_enc, E]
        encT = (enc_i.reshape(BC, t_enc, 2, 128).transpose(3, 1, 2, 0)
                .reshape(128, t_enc * 128))
        m = dict(shared)
        m["encT"] = np.ascontiguousarray(encT)
        m["h0T"] = _feat_major(h0[sl], 4).astype(bf16)
        in_maps.append(m)
    return in_maps


def kernel(encoder_outputs, encoder_hidden, emb_W, emb_b, w_ih, w_hh,
           b_ih, b_hh, out_W, out_b, reg_W, reg_b):
    from concourse.bass_utils import run_bass_kernel_spmd

    nc = build_program()
    in_maps = prep_inputs(encoder_outputs, encoder_hidden, emb_W, emb_b,
                          w_ih, w_hh, b_ih, b_hh, out_W, out_b, reg_W, reg_b)
    res = run_bass_kernel_spmd(nc, in_maps, core_ids=list(range(NCORES)))
    out = np.empty((B, T_ALL, O), np.float32)
    for i in range(NCORES):
        out[i * BC:(i + 1) * BC] = res.results[i]["y"]
    return out


# revision 15
# speedup vs baseline: 1.1775x; 1.1775x over previous
"""Trainium2 Bass kernel for nn_DecoderRNN (GRU decoder, 140 sequential steps).

Strategy (data-parallel, per sharding hint): B=512 sharded 8 ways -> 64
batch rows/core; weights replicated. Feature-major on-chip layout
([128 part, chunk*64+batch] free).

v2 redesign vs baseline (1.33 ms):
  - Weo fusion: e_t = relu(Weo h_t + b_eo) with Weo = emb_W @ out_W
    precomputed on host; the out-projection/x intermediate disappears from
    the recurrence. Decoder outputs come from regWo = reg_W @ out_W.
  - Gate-major PSUM banks: E|R in bank A, Z|HN in bank B, IN|Y in bank C
    (each gate region [128, 4*64]). One wide ACT/DVE op per gate instead
    of 4 narrow ones (per-op overhead dominated at [128,64]).
  - Biases pre-added into PSUM with one K=4 "indicator" matmul per gate
    region (stationary = 4 bias chunks as rows, moving = 0/1 selector).
    Frees ACT ops of per-chunk bias and doubles as PE filler.
  - h' = (1-z)*n + z*h with omz/zh computed on DVE during the tanh window
    -> only 2 DVE ops after tanh on the critical path.
  - PE stream ordered: chain matmuls first, then y-projections / next
    step's bias MMs / dummy warmers as filler so the PE never idles long
    enough for HAM to re-throttle the clock to 1.2 GHz (baseline spent
    73% of the kernel at half clock).
"""

import numpy as np
import ml_dtypes

B, T_ENC, E, H, O, PRED_LEN = 512, 140, 256, 512, 64, 140
NCORES = 8
BC = B // NCORES           # 64 batch rows per core
T_ALL = T_ENC + PRED_LEN   # 280

bf16 = ml_dtypes.bfloat16


def _pack_tiles(wT, n_k, n_m):
    """Pack a [K, M] (pre-transposed) weight into [128, n_m*n_k*128] bf16:
    tile (mi, k) at cols (mi*n_k + k)*128."""
    K, M = wT.shape
    assert K == n_k * 128 and M == n_m * 128
    t = wT.reshape(n_k, 128, n_m, 128).transpose(2, 0, 1, 3)  # [mc, kc, 128, 128]
    t = t.transpose(2, 0, 1, 3).reshape(128, -1)
    return np.ascontiguousarray(t.astype(bf16))


def _feat_major(x, n_chunks):
    """[B, F] -> [128, n_chunks*B] feature-major chunk layout."""
    b, f = x.shape
    assert f == n_chunks * 128
    t = x.reshape(b, n_chunks, 128).transpose(2, 1, 0).reshape(128, n_chunks * b)
    return np.ascontiguousarray(t)


def build_program(nsteps=PRED_LEN, t_enc=T_ENC, lowering=True):
    """Build the Bass program (per-core SPMD). Returns nc."""
    import concourse.bass as bass
    import concourse.tile as tile
    from concourse import bacc, mybir

    AF = mybir.ActivationFunctionType
    OP = mybir.AluOpType
    f32 = mybir.dt.float32
    bf = mybir.dt.bfloat16

    t_all = t_enc + nsteps

    if lowering:
        nc = bacc.Bacc("TRN2", target_bir_lowering=True, debug=False)
    else:
        nc = bass.Bass("TRN2", target_bir_lowering=False, debug=False)

    # ---- DRAM I/O ----
    encT_d = nc.dram_tensor("encT", [128, t_enc * 128], bf, kind="ExternalInput").ap()
    h0_d = nc.dram_tensor("h0T", [128, 4 * BC], bf, kind="ExternalInput").ap()
    wih_d = nc.dram_tensor("wihT", [128, 48 * 128], bf, kind="ExternalInput").ap()
    whh_d = nc.dram_tensor("whhT", [128, 48 * 128], bf, kind="ExternalInput").ap()
    weo_d = nc.dram_tensor("weoT", [128, 16 * 128], bf, kind="ExternalInput").ap()
    emb_d = nc.dram_tensor("embT", [128, 8 * 128], bf, kind="ExternalInput").ap()
    regw_d = nc.dram_tensor("regwT", [128, 2 * O], bf, kind="ExternalInput").ap()
    regwo_d = nc.dram_tensor("regwoT", [128, 4 * O], bf, kind="ExternalInput").ap()
    biasst_d = nc.dram_tensor("biasst", [4, 6 * 128], bf, kind="ExternalInput").ap()
    ind_d = nc.dram_tensor("ind", [4, 256], bf, kind="ExternalInput").ap()
    breg_d = nc.dram_tensor("b_reg", [BC, O], f32, kind="ExternalInput").ap()
    bregd_d = nc.dram_tensor("b_regd", [BC, O], f32, kind="ExternalInput").ap()
    y_d = nc.dram_tensor("y", [BC, t_all, O], f32, kind="ExternalOutput").ap()
    dbg_d = nc.dram_tensor("dbg", [1, 1], f32, kind="ExternalOutput").ap()

    # bias bank order in biasst: E0(b_e), E1(b_eo), R, Z, HN, IN
    BE0, BE1, BR, BZ, BHN, BIN = range(6)

    with tile.TileContext(nc) as tc:
        import contextlib
        with contextlib.ExitStack() as ctx:
            consts = ctx.enter_context(tc.tile_pool(name="consts", bufs=1))
            temps = ctx.enter_context(tc.tile_pool(name="temps", bufs=3))
            ytmp = ctx.enter_context(tc.tile_pool(name="ytmp", bufs=4))
            psum = ctx.enter_context(tc.tile_pool(name="psum", bufs=2, space="PSUM"))

            # ---- ACT table warmup (walrus inserts the table load before the
            # first ACTIVATE; pin it to dependency-light dummy ops) ----
            wt = consts.tile([128, 8], f32, tag="wtbl", name="wtbl")
            nc.vector.memset(wt[:, 0:4], 0.0)
            nc.scalar.activation(wt[:, 4:5], wt[:, 0:1], AF.Relu)
            nc.scalar.activation(wt[:, 5:6], wt[:, 1:2], AF.Sigmoid)
            nc.scalar.activation(wt[:, 6:7], wt[:, 2:3], AF.Tanh)

            # ---- load constants into SBUF ----
            wih_sb = consts.tile([128, 48 * 128], bf, tag="wih")
            whh_sb = consts.tile([128, 48 * 128], bf, tag="whh")
            weo_sb = consts.tile([128, 16 * 128], bf, tag="weo")
            emb_sb = consts.tile([128, 8 * 128], bf, tag="emb")
            regw_sb = consts.tile([128, 2 * O], bf, tag="regw")
            regwo_sb = consts.tile([128, 4 * O], bf, tag="regwo")
            biasst_sb = consts.tile([4, 6 * 128], bf, tag="biasst")
            ind_sb = consts.tile([4, 256], bf, tag="ind")
            breg_sb = consts.tile([BC, O], f32, tag="breg")
            bregd_sb = consts.tile([BC, O], f32, tag="bregd")
            encT_sb = consts.tile([128, t_enc * 128], bf, tag="encT")

            nc.sync.dma_start(out=biasst_sb, in_=biasst_d)
            nc.sync.dma_start(out=ind_sb, in_=ind_d)
            nc.sync.dma_start(out=emb_sb, in_=emb_d)
            # x0 block (last encoder token) first so step 0 can start early
            lastblk = slice((t_enc - 1) * 128, t_enc * 128)
            nc.sync.dma_start(out=encT_sb[:, lastblk], in_=encT_d[:, lastblk])
            nc.sync.dma_start(out=whh_sb, in_=whh_d)
            nc.sync.dma_start(out=wih_sb, in_=wih_d)
            nc.sync.dma_start(out=weo_sb, in_=weo_d)
            nc.sync.dma_start(out=regw_sb, in_=regw_d)
            nc.sync.dma_start(out=regwo_sb, in_=regwo_d)
            nc.sync.dma_start(out=breg_sb, in_=breg_d)
            nc.sync.dma_start(out=bregd_sb, in_=bregd_d)
            # rest of encT in 4 chunks
            nsplit = 4
            per = (t_enc - 1) // nsplit + 1
            for i in range(nsplit):
                lo, hi = i * per, min((i + 1) * per, t_enc - 1)
                if lo >= hi:
                    continue
                nc.sync.dma_start(out=encT_sb[:, lo * 128:hi * 128],
                                  in_=encT_d[:, lo * 128:hi * 128])

            # ---- persistent state: h ping-pong [128, 4*BC] ----
            h_pp = [consts.tile([128, 4 * BC], bf, tag=f"h{s}", name=f"h{s}")
                    for s in range(2)]
            nc.sync.dma_start(out=h_pp[0], in_=h0_d)

            def wtile(sb, mi, k, n_k):
                j = (mi * n_k + k) * 128
                return sb[:, j:j + 128]

            def bias_mm(region, bank):
                st = biasst_sb[:, bank * 128:(bank + 1) * 128]
                nc.tensor.matmul(region, st, ind_sb, start=True, stop=False)

            def alloc_banks(t, n_dummy=0):
                """Allocate step-t PSUM banks; issue HAM-warmer dummies (into
                the E corner, pre-bias so they get overwritten) + bias MMs.

                Separate tiles per gate region so no gate read shares a tile
                with unrelated later writers (keeps the auto-semaphores
                tight). Y projections ride in the Z tile's tail columns.
                """
                psA = psum.tile([128, 512], f32, tag="A", name=f"psA{t % 2}")
                psB = psum.tile([128, 512], f32, tag="B", name=f"psB{t % 2}")
                psHN = psB[:, 0:256]
                psIN = psB[:, 256:512]
                psZY = psum.tile([128, 384], f32, tag="ZY", name=f"psZY{t % 2}")
                for _ in range(n_dummy):
                    # keeps the PE's activity monitor fed during the gate
                    # window (else it re-throttles the clock to 1.2 GHz)
                    nc.tensor.matmul(psA[0:1, 0:64], whh_sb[:, 0:1],
                                     encT_sb[:, 0:64], start=True, stop=True)
                bias_mm(psA[:, 0:256], BE0 if t == 0 else BE1)   # E
                bias_mm(psA[:, 256:512], BR)                     # R
                bias_mm(psZY[:, 0:256], BZ)                      # Z
                bias_mm(psHN, BHN)                               # HN
                bias_mm(psIN, BIN)                               # IN
                return psA, psHN, psIN, psZY

            banks = alloc_banks(0)

            for t in range(nsteps):
                p, pn = t % 2, (t + 1) % 2
                h_cur = h_pp[p]
                psA, psHN, psIN, psZY = banks

                # ---- dense PE block (chain order) ----
                # 1) E: e pre-act = Weo h + b_eo  (t=0: emb x0 + b_e)
                if t == 0:
                    for m in range(4):
                        for k in range(2):
                            nc.tensor.matmul(
                                psA[:, m * BC:(m + 1) * BC],
                                wtile(emb_sb, m, k, 2),
                                encT_sb[:, (t_enc - 1) * 128 + k * BC:
                                        (t_enc - 1) * 128 + (k + 1) * BC],
                                start=False, stop=(k == 1))
                else:
                    for m in range(4):
                        for k in range(4):
                            nc.tensor.matmul(
                                psA[:, m * BC:(m + 1) * BC],
                                wtile(weo_sb, m, k, 4),
                                h_cur[:, k * BC:(k + 1) * BC],
                                start=False, stop=(k == 3))
                # 2) e: one wide relu (bias already in PSUM via bias MM)
                e = temps.tile([128, 256], bf, tag="e")
                nc.scalar.activation(e, psA[:, 0:256], AF.Relu)
                # 3) gate regions ordered by remaining-chain length:
                #    R first (r -> t3 -> t4 -> tanh -> q -> h'), then HN (t3),
                #    IN (t4), Z last (z -> omz/zh -> q/h').
                def whh_mms(base, ps, col0, stop):
                    for m in range(4):
                        for k in range(4):
                            nc.tensor.matmul(
                                ps[:, col0 + m * BC:col0 + (m + 1) * BC],
                                wtile(whh_sb, base + m, k, 4),
                                h_cur[:, k * BC:(k + 1) * BC],
                                start=False, stop=(stop and k == 3))

                def wih_mms(base, ps, col0):
                    # k-outer: the last e chunk only gates 4 trailing MMs
                    for k in range(4):
                        for m in range(4):
                            nc.tensor.matmul(
                                ps[:, col0 + m * BC:col0 + (m + 1) * BC],
                                wtile(wih_sb, base + m, k, 4),
                                e[:, k * BC:(k + 1) * BC],
                                start=False, stop=(k == 3))

                whh_mms(0, psA, 256, False)    # R (whh part)
                wih_mms(0, psA, 256)           # R (wih part) - waits on e
                whh_mms(8, psHN, 0, True)      # HN (complete)
                wih_mms(8, psIN, 0)            # IN
                whh_mms(4, psZY, 0, False)     # Z (whh part)
                wih_mms(4, psZY, 0)            # Z (wih part)

                # ---- PE filler MMs: y projections, next banks' bias MMs,
                # warmers. No pending deps -> they execute while ACT/DVE run
                # the gate chain. Their DVE/DMA tails are emitted after the
                # gate ops so they can't head-of-line-block the DVE queue.
                if t > 0:
                    for k in range(4):
                        nc.tensor.matmul(psZY[:BC, 320:384],
                                         h_cur[:, k * BC:(k + 1) * BC],
                                         regwo_sb[:, k * O:(k + 1) * O],
                                         start=(k == 0), stop=(k == 3))
                if t < t_enc:
                    for k in range(2):
                        nc.tensor.matmul(psZY[:BC, 256:320],
                                         encT_sb[:, t * 128 + k * BC:
                                                 t * 128 + (k + 1) * BC],
                                         regw_sb[:, k * O:(k + 1) * O],
                                         start=(k == 0), stop=(k == 1))
                if t + 1 < nsteps:
                    banks = alloc_banks(t + 1, n_dummy=2)

                # ---- gate chain (ACT / DVE / GpSimd) ----
                r = temps.tile([128, 256], bf, tag="r")
                z = temps.tile([128, 256], bf, tag="z")
                t3 = temps.tile([128, 256], bf, tag="t3")
                t4 = temps.tile([128, 256], bf, tag="t4")
                n_t = temps.tile([128, 256], bf, tag="n")
                omz = temps.tile([128, 256], bf, tag="omz")
                zh = temps.tile([128, 256], bf, tag="zh")
                q = temps.tile([128, 256], bf, tag="q")
                h_nxt = h_pp[pn]

                # ACT queue: relu(above), r, tanh, z
                nc.scalar.activation(r, psA[:, 256:512], AF.Sigmoid)
                nc.vector.tensor_tensor(t3, psHN, r, OP.mult)
                nc.vector.tensor_tensor(t4, t3, psIN, OP.add)
                nc.scalar.activation(n_t, t4, AF.Tanh)
                nc.scalar.activation(z, psZY[:, 0:256], AF.Sigmoid)
                nc.vector.tensor_scalar(omz, z, -1.0, 1.0, OP.mult, OP.add)
                nc.gpsimd.tensor_tensor(zh, z, h_cur, OP.mult)
                nc.vector.tensor_tensor(q, omz, n_t, OP.mult)
                nc.vector.tensor_tensor(h_nxt, q, zh, OP.add)

                # chain-gated HAM warmers: each becomes ready only as the
                # gate chain progresses, spreading PE activity through the
                # tail so the activity monitor never sees an idle window.
                # They write row 0 of the E region (already consumed by
                # relu; next step's bias MM start=True overwrites).
                for src_t in (r, t3, t4, z, n_t, q):
                    nc.tensor.matmul(psA[0:1, 0:64], src_t[:, 0:1],
                                     encT_sb[:, 0:64], start=True, stop=True)

                # ---- y staging (DVE) + DMA, after the gate chain on DVE ----
                if t > 0:
                    y_dec = ytmp.tile([BC, O], f32, tag="ydec")
                    nc.vector.tensor_tensor(y_dec, psZY[:BC, 320:384],
                                            bregd_sb, OP.add)
                    nc.sync.dma_start(out=y_d[:, t_enc + t - 1, :], in_=y_dec)
                if t < t_enc:
                    y_enc = ytmp.tile([BC, O], f32, tag="yenc")
                    nc.vector.tensor_tensor(y_enc, psZY[:BC, 256:320],
                                            breg_sb, OP.add)
                    nc.sync.dma_start(out=y_d[:, t, :], in_=y_enc)

            # final decoder token: dec-y for h_{nsteps} -> y[t_enc+nsteps-1]
            h_last = h_pp[nsteps % 2]
            ps_fin = psum.tile([128, 512], f32, tag="A", name="psfin")
            for k in range(4):
                nc.tensor.matmul(ps_fin[:BC, 320:384],
                                 h_last[:, k * BC:(k + 1) * BC],
                                 regwo_sb[:, k * O:(k + 1) * O],
                                 start=(k == 0), stop=(k == 3))
            y_fin = ytmp.tile([BC, O], f32, tag="ydec")
            nc.vector.tensor_tensor(y_fin, ps_fin[:BC, 320:384], bregd_sb, OP.add)
            nc.sync.dma_start(out=y_d[:, t_enc + nsteps - 1, :], in_=y_fin)

            # leftover encoder tokens if nsteps < t_enc (smoke tests only)
            for t in range(nsteps, t_enc):
                ps_y2 = psum.tile([128, 512], f32, tag="A", name=f"psy{t}")
                for k in range(2):
                    nc.tensor.matmul(ps_y2[:BC, 256:320],
                                     encT_sb[:, t * 128 + k * BC:
                                             t * 128 + (k + 1) * BC],
                                     regw_sb[:, k * O:(k + 1) * O],
                                     start=(k == 0), stop=(k == 1))
                y_enc = ytmp.tile([BC, O], f32, tag="yenc")
                nc.vector.tensor_tensor(y_enc, ps_y2[:BC, 256:320], breg_sb, OP.add)
                nc.sync.dma_start(out=y_d[:, t, :], in_=y_enc)

            # anti-DCE sink for the warmer scratch
            ps_wm = psum.tile([128, 256], f32, tag="wm", bufs=1, name="wmfin")
            nc.tensor.matmul(ps_wm, whh_sb[:, 0:128], encT_sb[:, 0:256],
                             start=True, stop=True)
            wm_sb = ytmp.tile([1, 1], f32, tag="wmsb")
            nc.vector.tensor_copy(wm_sb, ps_wm[0:1, 0:1])
            nc.sync.dma_start(out=dbg_d, in_=wm_sb)

    if lowering:
        nc.finalize()
    return nc


def prep_inputs(encoder_outputs, encoder_hidden, emb_W, emb_b, w_ih, w_hh,
                b_ih, b_hh, out_W, out_b, reg_W, reg_b, nsteps=PRED_LEN,
                t_enc=T_ENC):
    """Host-side packing. Returns per-core input dicts."""
    f32 = np.float32
    emb_W, emb_b, w_ih, w_hh, b_ih, b_hh, out_W, out_b, reg_W, reg_b = (
        np.asarray(a, f32) for a in
        (emb_W, emb_b, w_ih, w_hh, b_ih, b_hh, out_W, out_b, reg_W, reg_b))

    weo = emb_W @ out_W            # [H, H]
    b_eo = emb_W @ out_b + emb_b   # [H]
    regwo = reg_W @ out_W          # [O, H]
    b_regd = reg_W @ out_b + reg_b # [O]

    # bias stationary [4, 6*128]: rows = feature chunks; bank order
    # E0(b_e), E1(b_eo), R, Z, HN, IN
    b_r = b_ih[0:H] + b_hh[0:H]
    b_z = b_ih[H:2 * H] + b_hh[H:2 * H]
    b_hn = b_hh[2 * H:]
    b_in = b_ih[2 * H:]
    biasst = np.stack([emb_b, b_eo, b_r, b_z, b_hn, b_in], 0)  # [6, 512]
    biasst = (biasst.reshape(6, 4, 128).transpose(1, 0, 2)
              .reshape(4, 6 * 128))
    ind = np.zeros((4, 256), f32)
    for c in range(4):
        ind[c, c * BC:(c + 1) * BC] = 1.0

    shared = {
        "wihT": _pack_tiles(w_ih.T, 4, 12),
        "whhT": _pack_tiles(w_hh.T, 4, 12),
        "weoT": _pack_tiles(weo.T, 4, 4),
        "embT": _pack_tiles(emb_W.T, 2, 4),
        "regwT": np.ascontiguousarray(
            reg_W.T.reshape(2, 128, O).transpose(1, 0, 2).reshape(128, 2 * O)
            .astype(bf16)),
        "regwoT": np.ascontiguousarray(
            regwo.T.reshape(4, 128, O).transpose(1, 0, 2).reshape(128, 4 * O)
            .astype(bf16)),
        "biasst": np.ascontiguousarray(biasst.astype(bf16)),
        "ind": np.ascontiguousarray(ind.astype(bf16)),
        "b_reg": np.ascontiguousarray(np.tile(reg_b[None, :], (BC, 1)).astype(f32)),
        "b_regd": np.ascontiguousarray(np.tile(b_regd[None, :], (BC, 1)).astype(f32)),
    }

    enc = np.asarray(encoder_outputs, f32)[:, :t_enc, :]
    h0 = np.asarray(encoder_hidden, f32)[0]
    in_maps = []
    for i in range(NCORES):
        sl = slice(i * BC, (i + 1) * BC)
        enc_i = enc[sl].astype(bf16)              # [BC, t_enc, E]
        encT = (enc_i.reshape(BC, t_enc, 2, 128).transpose(3, 1, 2, 0)
                .reshape(128, t_enc * 128))
        m = dict(shared)
        m["encT"] = np.ascontiguousarray(encT)
        m["h0T"] = _feat_major(h0[sl], 4).astype(bf16)
        in_maps.append(m)
    return in_maps


def kernel(encoder_outputs, encoder_hidden, emb_W, emb_b, w_ih, w_hh,
           b_ih, b_hh, out_W, out_b, reg_W, reg_b):
    from concourse.bass_utils import run_bass_kernel_spmd

    nc = build_program()
    in_maps = prep_inputs(encoder_outputs, encoder_hidden, emb_W, emb_b,
                          w_ih, w_hh, b_ih, b_hh, out_W, out_b, reg_W, reg_b)
    res = run_bass_kernel_spmd(nc, in_maps, core_ids=list(range(NCORES)))
    out = np.empty((B, T_ALL, O), np.float32)
    for i in range(NCORES):
        out[i * BC:(i + 1) * BC] = res.results[i]["y"]
    return out
